# revision 41
# baseline (speedup 1.0000x reference)
"""Trainium2 Bass kernel for fused multi-head attention (B=4, N=2048, D=384, h=8, dh=48).

Sharding: 32 (batch, head) pairs across 8 cores -> core c handles batch c//2 and
heads [4*(c%2), 4*(c%2)+4). Each core computes a *partial* output projection
(its 4 heads' contribution to out @ Wproj); the host sums the two partials per
batch and adds bproj.

Per-core algorithm (everything in "transposed" layout so no PE transposes are
needed):
  xT   [384, 2048]  = x^T                          (transposed on host, bf16)
  QT   [256, 2048]  = (Wq_pad)^T @ xT  (4 heads padded dh 48->64, pair-packed)
  KT   same
  V'   [2048, 4*64] = x @ Wv_pad, with a ones-column per head at col h*64+32
                      (gpsimd memset) -> the PV matmul also accumulates the
                      softmax denominator Z for free.
  attention, software-pipelined at 512-query granularity: per (pair, q-half,
  head, key-chunk kc, j-half):
      simT[k, q512] = KT_h^T @ QT_h   (PSUM [128,512] = 1 bank, 4-deep ring;
                                       sims run 2 kc-steps ahead of PV)
      E = exp(simT)  j=0 -> ACT engine (exact exp, psum->sbuf bf16)
                     j=1 -> DVE via Schraudolph bit-trick: i16 = s*A + B
                            bitcast to bf16 ~= exp(s) (+-4% per weight, which
                            softmax-averages to ~2e-3 on the final output;
                            scores ~N(0,1) so no max subtraction needed)
      acc[o:o+64, q] += V'_kc^T @ E   (PSUM accumulate; row o+32 = Z)
  OT_h = acc[o:o+64] * (1/Z)          (DVE approx-reciprocal + gpsimd partition
                                       broadcast + DVE multiply), bf16
  y    [2048, 384]  = sum_h OT_h^T @ Wproj_h   (partial; f32 out; first half
                      interleaved into the third attention block)
"""

import os

os.environ.pop("JAX_PLATFORMS", None)  # the bass PJRT path needs the axon platform

import numpy as np
import ml_dtypes

import concourse.mybir as mybir
import concourse.tile as tile
from concourse import bacc
from concourse.bass_utils import run_bass_kernel_spmd

BF16 = ml_dtypes.bfloat16

# problem shapes (hardcoded per contract)
B, N, D = 4, 2048, 384
H, DH = 8, 48
SCALE = DH**-0.5
N_CORES = 8
HP = 4  # heads per core
DHP = 64  # padded head dim
P = 128
NKC = N // P  # 16 key-row chunks
ZOFF = 32  # partition offset of the fused softmax-denominator (Z) row within a
# head's 64-row block: engines need 32-aligned partition starts, so the ones
# column sits at col 32 of each head's V' block; v-dims occupy cols
# [0,32) and [33,49), the rest are zero. Wproj rows are laid out to match,
# with zeros at the Z/pad rows.

# Schraudolph exp for the DVE half: exp(s) ~= bitcast_bf16(int16(s*A + B)).
# A = 128/ln2; B = 127*128 + delta with delta centering the piecewise-linear
# 2^frac ~ 1+frac error (worst case ~+-4%, validated end-to-end offline).
SCHR_A = 128.0 / 0.6931471805599453
SCHR_B = 16248.87

LAST_EXEC_NS = None
_CACHE = {}


def _build_bass():
    f32 = mybir.dt.float32
    bf16 = mybir.dt.bfloat16
    i16 = mybir.dt.int16
    EXP = mybir.ActivationFunctionType.Exp
    MULT = mybir.AluOpType.mult
    ADD = mybir.AluOpType.add

    nc = bacc.Bacc("TRN2", target_bir_lowering=False, debug=False, num_devices=N_CORES)
    xbT = nc.dram_tensor("xbT", [D, N], bf16, kind="ExternalInput").ap()
    wq = nc.dram_tensor("wq", [D, HP * DHP], bf16, kind="ExternalInput").ap()
    wk = nc.dram_tensor("wk", [D, HP * DHP], bf16, kind="ExternalInput").ap()
    wv = nc.dram_tensor("wv", [D, HP * DHP], bf16, kind="ExternalInput").ap()
    wpj = nc.dram_tensor("wpj", [2, P, D], bf16, kind="ExternalInput").ap()
    # bf16 partials (summed in f32 on the host): halves the output DMA bytes
    y = nc.dram_tensor("y", [N, D], bf16, kind="ExternalOutput").ap()

    with tile.TileContext(nc) as tc:
        with (
            tc.tile_pool(name="const", bufs=1) as cpool,
            tc.tile_pool(name="epool", bufs=8) as epool,
            tc.tile_pool(name="rpool", bufs=3) as rpool,
            tc.tile_pool(name="ysb", bufs=6) as ypool,
            tc.tile_pool(name="simps", bufs=4, space="PSUM") as simps,
            tc.tile_pool(name="accps", bufs=2, space="PSUM") as accps,
            tc.tile_pool(name="auxps", bufs=2, space="PSUM") as auxps,
        ):
            # ---- load weights / x ----
            # 4-queue DMA plan (22.5 B/ns each): wk chunk + x column-halves per
            # HWDGE queue; wq (sims need it early), wv, wpj on the gpsimd
            # SWDGE queue in need order.
            wq_sb, wk_sb, wv_sb = [], [], []
            for name, srct, dst in (("wk", wk, wk_sb), ("wq", wq, wq_sb), ("wv", wv, wv_sb)):
                for i in range(3):
                    t = cpool.tile([P, HP * DHP], bf16, name=f"{name}{i}", tag=f"{name}{i}")
                    dst.append(t)
            xT = [cpool.tile([P, N], bf16, name=f"xT{i}", tag=f"xT{i}") for i in range(3)]
            wpj_sb = [
                cpool.tile([P, D], bf16, name=f"wpj{p}", tag=f"wpj{p}") for p in range(2)
            ]
            # per-dk queue: sync=dk0, scalar=dk1, gpsimd=dk2; weight chunks in
            # need order, x in quarter-column chunks so the attention pipeline
            # can start on quarter 0 while the rest streams
            xq = [nc.sync, nc.scalar, nc.gpsimd]
            for i in range(3):
                xq[i].dma_start(out=wk_sb[i][:], in_=wk[i * P : (i + 1) * P, :])
            for i in range(3):
                xq[i].dma_start(out=wq_sb[i][:], in_=wq[i * P : (i + 1) * P, :])
            for i in range(3):
                xq[i].dma_start(
                    out=xT[i][:, 0:512], in_=xbT[i * P : (i + 1) * P, 0:512]
                )
            for i in range(3):
                xq[i].dma_start(out=wv_sb[i][:], in_=wv[i * P : (i + 1) * P, :])
            for q_ in range(1, 4):
                for i in range(3):
                    xq[i].dma_start(
                        out=xT[i][:, q_ * 512 : (q_ + 1) * 512],
                        in_=xbT[i * P : (i + 1) * P, q_ * 512 : (q_ + 1) * 512],
                    )
            for p in range(2):
                nc.gpsimd.dma_start(out=wpj_sb[p][:], in_=wpj[p])

            # ---- QKV projection (chunked; most of it interleaved into the
            # attention pipeline as data arrives) ----
            QT = [cpool.tile([P, N], bf16, name=f"QT{p}", tag=f"QT{p}") for p in range(2)]
            KT = [cpool.tile([P, N], bf16, name=f"KT{p}", tag=f"KT{p}") for p in range(2)]
            # all 16 V chunks side by side: pair-production needs one copy per
            # two chunks; V chunk kc lives at cols [kc*256, (kc+1)*256)
            Vall = cpool.tile([P, NKC * HP * DHP], bf16, name="Vall", tag="Vall")
            qkv_alt = [0]  # copy-engine alternator

            def emit_qk_chunk(p, gi, j, in_attn=True):
                w_sb, dstl = ((wk_sb, KT), (wq_sb, QT))[gi]
                a = qkv_alt[0] = qkv_alt[0] + 1
                # attention-time productions use dedicated PSUM banks so they
                # never squat a sim-ring slot (that shrinks the pipeline and
                # drops the PE out of its fast p-state)
                pool, tg = (auxps, "aux") if in_attn else (simps, "sim")
                ps = pool.tile([P, 512], f32, name="qkvps", tag=tg)
                for dk in range(3):
                    nc.tensor.matmul(
                        ps[:],
                        lhsT=w_sb[dk][:, p * P : (p + 1) * P],
                        rhs=xT[dk][:, j * 512 : (j + 1) * 512],
                        start=(dk == 0),
                        stop=(dk == 2),
                    )
                if a % 2 == 0:
                    nc.vector.tensor_copy(dstl[p][:, j * 512 : (j + 1) * 512], ps[:])
                else:
                    nc.scalar.copy(dstl[p][:, j * 512 : (j + 1) * 512], ps[:])

            def emit_vpair(k2, in_attn=True):
                a = qkv_alt[0] = qkv_alt[0] + 1
                pool, tg = (auxps, "aux") if in_attn else (simps, "sim")
                ps = pool.tile([P, 2 * HP * DHP], f32, name="vps", tag=tg)
                for c in (0, 1):
                    kc = 2 * k2 + c
                    for dk in range(3):
                        nc.tensor.matmul(
                            ps[:, c * 256 : (c + 1) * 256],
                            lhsT=xT[dk][:, kc * P : (kc + 1) * P],
                            rhs=wv_sb[dk][:],
                            start=(dk == 0),
                            stop=(dk == 2),
                        )
                dst = Vall[:, k2 * 512 : (k2 + 1) * 512]
                if a % 2 == 0:
                    nc.vector.tensor_copy(dst, ps[:])
                else:
                    nc.scalar.copy(dst, ps[:])
                # ones (Z) column of each head block, at col h*64+ZOFF
                zcols = dst.rearrange("p (g c) -> p g c", c=DHP)[:, :, ZOFF : ZOFF + 1]
                nc.gpsimd.memset(zcols, 1.0)

            # preamble: productions the prologue sims need (x quarter 0 only)
            emit_qk_chunk(0, 0, 0, in_attn=False)  # KT[0] quarter 0
            emit_qk_chunk(0, 1, 0, in_attn=False)  # QT[0] query-quarter 0

            # ---- attention: 8 blocks of (pair, 512-query-range), hh-inner so
            # each x chunk feeds two consecutive half-steps ----
            OT = [cpool.tile([P, N], bf16, name=f"OT{p}", tag=f"OT{p}") for p in range(2)]
            BLOCKS8 = [(p, qq) for qq in range(4) for p in (0, 1)]
            seq = []
            for p, qq in BLOCKS8:
                for kc in range(NKC):
                    for hh in (0, 1):
                        seq.append((p, qq, hh, kc))
            nh = len(seq)  # 256
            HLOOK = 4  # half-steps the sims run ahead of the PVs

            # auto-schedule each qkv production chunk right before its first
            # consumer in the pipeline (sims run HLOOK ahead)
            side = {}

            def addside(s_, fn):
                side.setdefault(s_, []).append(fn)

            def even(i):  # iterations advance two half-steps at a time
                return max(0, (i // 2) * 2)

            first_k, first_q, first_v = {}, {}, {}
            for idx, (p, qq, hh, kc) in enumerate(seq):
                first_k.setdefault((p, kc // 4), idx)
                first_q.setdefault((p, qq), idx)
                first_v.setdefault(kc // 2, idx)
            for (p, qr), idx in first_k.items():
                if (p, qr) != (0, 0):
                    addside(even(idx - HLOOK - 1), (lambda pp, q_: lambda: emit_qk_chunk(pp, 0, q_))(p, qr))
            for (p, qj), idx in first_q.items():
                if (p, qj) != (0, 0):
                    addside(even(idx - HLOOK - 1), (lambda pp, q_: lambda: emit_qk_chunk(pp, 1, q_))(p, qj))
            for k2, idx in first_v.items():
                addside(even(idx - 2), (lambda i: lambda: emit_vpair(i))(k2))

            accs = {}
            es = {}

            def emit_sim(hi):
                p, qq, hh, kc = seq[hi]
                o = hh * DHP
                sp = simps.tile([P, 512], f32, name="sim", tag="sim")
                nc.tensor.matmul(
                    sp[:],
                    lhsT=KT[p][o : o + DHP, kc * P : (kc + 1) * P],
                    rhs=QT[p][o : o + DHP, qq * 512 : (qq + 1) * 512],
                    start=True,
                    stop=True,
                )
                e = epool.tile([P, 512], bf16, name="E", tag="E")
                # 5/8 of tiles on ACT (exact exp), 3/8 on DVE (Schraudolph):
                # leaves the DVE slack to absorb the normalize work.
                if (2 * kc + hh) % 8 in (1, 4, 6):
                    nc.vector.tensor_scalar(e[:].bitcast(i16), sp[:], SCHR_A, SCHR_B, MULT, ADD)
                else:
                    nc.scalar.activation(e[:], sp[:], EXP)
                es[hi] = e

            def emit_pv(hi):
                p, qq, hh, kc = seq[hi]
                bi = hi // 32
                o = hh * DHP
                h = p * 2 + hh
                if bi not in accs:
                    # one 1-bank accumulator per block (2-deep ring)
                    accs[bi] = accps.tile([P, 512], f32, name="acc", tag="acc")
                acc = accs[bi]
                e = es.pop(hi)
                nc.tensor.matmul(
                    acc[o : o + DHP, :],
                    lhsT=Vall[:, kc * 256 + h * DHP : kc * 256 + (h + 1) * DHP],
                    rhs=e[:],
                    start=(kc == 0),
                    stop=(kc == NKC - 1),
                )

            def norm_phase0(bi, hh, tail=False):
                # r = 1/Z for one head chunk (Z at acc row o+ZOFF), broadcast
                # to a 64-partition R tile for the multiply
                acc = accs[bi]
                o = hh * DHP
                zrow = rpool.tile([1, 512], f32, name="zrow", tag="zrow")
                if tail:  # ACT is idle at the tail; shorten the DVE chain
                    nc.scalar.copy(zrow[:], acc[o + ZOFF : o + ZOFF + 1, :])
                else:
                    nc.vector.tensor_copy(zrow[:], acc[o + ZOFF : o + ZOFF + 1, :])
                r = rpool.tile([1, 512], f32, name="r", tag="r")
                nc.vector.reciprocal_approx_fast(r[:], zrow[:])
                R = rpool.tile([DHP, 512], f32, name="R", tag="R")
                nc.gpsimd.partition_broadcast(R[:], r[:], channels=DHP)
                return R

            def norm_phase1(bi, hh, R):
                # OT[o:o+64, chunk] = acc[o:o+64, chunk] * (1/Z)
                p, qq = BLOCKS8[bi]
                o = hh * DHP
                nc.vector.tensor_mul(
                    OT[p][o : o + DHP, qq * 512 : (qq + 1) * 512],
                    accs[bi][o : o + DHP, :],
                    R[:],
                )

            def norm_chunk(bi, hh, tail=False):
                norm_phase1(bi, hh, norm_phase0(bi, hh, tail))

            # ---- output projection (partial: this core's 4 heads) ----
            def emit_proj(mc):
                yp = auxps.tile([P, D], f32, name="yp", tag="aux")
                for p in range(2):  # K=128 covers both heads of the pair
                    nc.tensor.matmul(
                        yp[:],
                        lhsT=OT[p][:, mc * P : (mc + 1) * P],
                        rhs=wpj_sb[p][:],
                        start=(p == 0),
                        stop=(p == 1),
                    )
                ys = ypool.tile([P, D], bf16, name="ys", tag="ys")
                if mc % 2 == 0:
                    nc.vector.tensor_copy(ys[:], yp[:])
                else:
                    nc.scalar.copy(ys[:], yp[:])
                if mc < 12:
                    q = (nc.sync, nc.gpsimd)[mc % 2]
                    q.dma_start(out=y[mc * P : (mc + 1) * P, :], in_=ys[:])
                else:
                    # tail: split each row-block across two of the 3 queues
                    qs = (nc.sync, nc.gpsimd, nc.scalar)
                    for half in (0, 1):
                        c0, c1 = half * 192, (half + 1) * 192
                        qs[(2 * mc + half) % 3].dma_start(
                            out=y[mc * P : (mc + 1) * P, c0:c1], in_=ys[:, c0:c1]
                        )

            proj_pending = []  # mc's ready to emit, drained 1 per 8 half-iters
            norm_pending = []  # deferred normalize chunks, same cadence

            def drain_norm():
                bi, hh, phase, R = norm_pending.pop(0)
                if phase == 0:  # recip half; queue the mul next
                    norm_pending.insert(0, (bi, hh, 1, norm_phase0(bi, hh)))
                    return
                norm_phase1(bi, hh, R)
                if not any(n[0] == bi for n in norm_pending):
                    accs.pop(bi)
                    if bi % 2 == 1 and bi < 7:
                        # both pair-blocks of query-range bi//2 are normalized
                        proj_pending.extend(range(4 * (bi // 2), 4 * (bi // 2) + 4))

            for hi in range(HLOOK):
                emit_sim(hi)
            for hp in range(0, nh, 2):
                for fn in side.pop(hp, ()):
                    fn()
                for hi in (hp + HLOOK, hp + HLOOK + 1):
                    if hi < nh:
                        emit_sim(hi)
                emit_pv(hp)
                emit_pv(hp + 1)
                if proj_pending and hp % 8 == 0:
                    emit_proj(proj_pending.pop(0))
                if norm_pending and hp % 8 == 4:
                    drain_norm()
                if hp % 32 == 30:  # block boundary
                    bi = hp // 32
                    if bi < 7:
                        # defer both chunks into the next block (the acc ring
                        # is 2 blocks deep, and bursts here knock the PE out
                        # of its fast p-state)
                        norm_pending.extend([(bi, 0, 0, None), (bi, 1, 0, None)])
                    else:
                        while norm_pending:
                            drain_norm()
                        norm_chunk(7, 0, tail=True)
                        norm_chunk(7, 1, tail=True)
                        accs.pop(7)
                        for mc in range(12, 16):
                            emit_proj(mc)

    nc.compile()
    return nc


def _prep_core_inputs(x, Wqkv, Wproj, core):
    b, hg = core // 2, core % 2
    heads = [hg * HP + i for i in range(HP)]
    xbT = np.ascontiguousarray(x[b].astype(BF16).T)
    wq = np.zeros((D, HP * DHP), np.float32)
    wk = np.zeros((D, HP * DHP), np.float32)
    wv = np.zeros((D, HP * DHP), np.float32)
    wpj = np.zeros((2, P, D), np.float32)
    for i, h in enumerate(heads):
        wq[:, i * DHP : i * DHP + DH] = Wqkv[:, h * DH : (h + 1) * DH] * SCALE
        wk[:, i * DHP : i * DHP + DH] = Wqkv[:, H * DH + h * DH : H * DH + (h + 1) * DH]
        wv_h = Wqkv[:, 2 * H * DH + h * DH : 2 * H * DH + (h + 1) * DH]
        wpj_h = Wproj[h * DH : (h + 1) * DH, :]
        # v-dims at cols [0,ZOFF) and [ZOFF+1, DH+1); ones (Z) column at ZOFF
        wv[:, i * DHP : i * DHP + ZOFF] = wv_h[:, :ZOFF]
        wv[:, i * DHP + ZOFF + 1 : i * DHP + DH + 1] = wv_h[:, ZOFF:]
        o = (i % 2) * DHP
        wpj[i // 2, o : o + ZOFF, :] = wpj_h[:ZOFF, :]
        wpj[i // 2, o + ZOFF + 1 : o + DH + 1, :] = wpj_h[ZOFF:, :]
    return {
        "xbT": xbT,
        "wq": wq.astype(BF16),
        "wk": wk.astype(BF16),
        "wv": wv.astype(BF16),
        "wpj": wpj.astype(BF16),
    }


def kernel(x, Wqkv, Wproj, bproj):
    global LAST_EXEC_NS
    if "nc" not in _CACHE:
        _CACHE["nc"] = _build_bass()
    nc = _CACHE["nc"]
    in_maps = [_prep_core_inputs(x, Wqkv, Wproj, c) for c in range(N_CORES)]
    try:
        res = run_bass_kernel_spmd(nc, in_maps, core_ids=list(range(N_CORES)))
    except Exception:
        res = run_bass_kernel_spmd(nc, in_maps, core_ids=list(range(N_CORES)))
    LAST_EXEC_NS = res.exec_time_ns
    out = np.empty((B, N, D), np.float32)
    for b in range(B):
        out[b] = res.results[2 * b]["y"].astype(np.float32) + res.results[
            2 * b + 1
        ]["y"].astype(np.float32)
    out += bproj.astype(np.float32)[None, None, :]
    return out


# revision 42
# speedup vs baseline: 1.0158x; 1.0158x over previous
"""Trainium2 Bass kernel for fused multi-head attention (B=4, N=2048, D=384, h=8, dh=48).

Sharding: 32 (batch, head) pairs across 8 cores -> core c handles batch c//2 and
heads [4*(c%2), 4*(c%2)+4). Each core computes a *partial* output projection
(its 4 heads' contribution to out @ Wproj); the host sums the two partials per
batch and adds bproj.

Per-core algorithm (everything in "transposed" layout so no PE transposes are
needed):
  xT   [384, 2048]  = x^T                          (transposed on host, bf16)
  QT   [256, 2048]  = (Wq_pad)^T @ xT  (4 heads padded dh 48->64, pair-packed)
  KT   same
  V'   [2048, 4*64] = x @ Wv_pad, with a ones-column per head at col h*64+32
                      (gpsimd memset) -> the PV matmul also accumulates the
                      softmax denominator Z for free.
  attention, software-pipelined at 512-query granularity: per (pair, q-half,
  head, key-chunk kc, j-half):
      simT[k, q512] = KT_h^T @ QT_h   (PSUM [128,512] = 1 bank, 4-deep ring;
                                       sims run 2 kc-steps ahead of PV)
      E = exp(simT)  j=0 -> ACT engine (exact exp, psum->sbuf bf16)
                     j=1 -> DVE via Schraudolph bit-trick: i16 = s*A + B
                            bitcast to bf16 ~= exp(s) (+-4% per weight, which
                            softmax-averages to ~2e-3 on the final output;
                            scores ~N(0,1) so no max subtraction needed)
      acc[o:o+64, q] += V'_kc^T @ E   (PSUM accumulate; row o+32 = Z)
  OT_h = acc[o:o+64] * (1/Z)          (DVE approx-reciprocal + gpsimd partition
                                       broadcast + DVE multiply), bf16
  y    [2048, 384]  = sum_h OT_h^T @ Wproj_h   (partial; f32 out; first half
                      interleaved into the third attention block)
"""

import os

os.environ.pop("JAX_PLATFORMS", None)  # the bass PJRT path needs the axon platform

import numpy as np
import ml_dtypes

import concourse.mybir as mybir
import concourse.tile as tile
from concourse import bacc
from concourse.bass_utils import run_bass_kernel_spmd

BF16 = ml_dtypes.bfloat16

# problem shapes (hardcoded per contract)
B, N, D = 4, 2048, 384
H, DH = 8, 48
SCALE = DH**-0.5
N_CORES = 8
HP = 4  # heads per core
DHP = 64  # padded head dim
P = 128
NKC = N // P  # 16 key-row chunks
ZOFF = 32  # partition offset of the fused softmax-denominator (Z) row within a
# head's 64-row block: engines need 32-aligned partition starts, so the ones
# column sits at col 32 of each head's V' block; v-dims occupy cols
# [0,32) and [33,49), the rest are zero. Wproj rows are laid out to match,
# with zeros at the Z/pad rows.

# Schraudolph exp for the DVE half: exp(s) ~= bitcast_bf16(int16(s*A + B)).
# A = 128/ln2; B = 127*128 + delta with delta centering the piecewise-linear
# 2^frac ~ 1+frac error (worst case ~+-4%, validated end-to-end offline).
SCHR_A = 128.0 / 0.6931471805599453
SCHR_B = 16248.87

LAST_EXEC_NS = None
_CACHE = {}


def _build_bass():
    f32 = mybir.dt.float32
    bf16 = mybir.dt.bfloat16
    i16 = mybir.dt.int16
    EXP = mybir.ActivationFunctionType.Exp
    MULT = mybir.AluOpType.mult
    ADD = mybir.AluOpType.add

    nc = bacc.Bacc("TRN2", target_bir_lowering=False, debug=False, num_devices=N_CORES)
    xbT = nc.dram_tensor("xbT", [D, N], bf16, kind="ExternalInput").ap()
    wq = nc.dram_tensor("wq", [D, HP * DHP], bf16, kind="ExternalInput").ap()
    wk = nc.dram_tensor("wk", [D, HP * DHP], bf16, kind="ExternalInput").ap()
    wv = nc.dram_tensor("wv", [D, HP * DHP], bf16, kind="ExternalInput").ap()
    wpj = nc.dram_tensor("wpj", [2, P, D], bf16, kind="ExternalInput").ap()
    # bf16 partials (summed in f32 on the host): halves the output DMA bytes
    y = nc.dram_tensor("y", [N, D], bf16, kind="ExternalOutput").ap()

    with tile.TileContext(nc) as tc:
        with (
            tc.tile_pool(name="const", bufs=1) as cpool,
            tc.tile_pool(name="epool", bufs=8) as epool,
            tc.tile_pool(name="rpool", bufs=3) as rpool,
            tc.tile_pool(name="ysb", bufs=6) as ypool,
            tc.tile_pool(name="simps", bufs=4, space="PSUM") as simps,
            tc.tile_pool(name="accps", bufs=2, space="PSUM") as accps,
            tc.tile_pool(name="auxps", bufs=2, space="PSUM") as auxps,
        ):
            # ---- load weights / x ----
            # 4-queue DMA plan (22.5 B/ns each): wk chunk + x column-halves per
            # HWDGE queue; wq (sims need it early), wv, wpj on the gpsimd
            # SWDGE queue in need order.
            wq_sb, wk_sb, wv_sb = [], [], []
            for name, srct, dst in (("wk", wk, wk_sb), ("wq", wq, wq_sb), ("wv", wv, wv_sb)):
                for i in range(3):
                    t = cpool.tile([P, HP * DHP], bf16, name=f"{name}{i}", tag=f"{name}{i}")
                    dst.append(t)
            xT = [cpool.tile([P, N], bf16, name=f"xT{i}", tag=f"xT{i}") for i in range(3)]
            wpj_sb = [
                cpool.tile([P, D], bf16, name=f"wpj{p}", tag=f"wpj{p}") for p in range(2)
            ]
            # per-dk queue: sync=dk0, scalar=dk1, gpsimd=dk2; weight chunks in
            # need order, x in quarter-column chunks so the attention pipeline
            # can start on quarter 0 while the rest streams
            xq = [nc.sync, nc.scalar, nc.gpsimd]
            for i in range(3):
                xq[i].dma_start(out=wk_sb[i][:], in_=wk[i * P : (i + 1) * P, :])
            for i in range(3):
                xq[i].dma_start(out=wq_sb[i][:], in_=wq[i * P : (i + 1) * P, :])
            for i in range(3):
                xq[i].dma_start(
                    out=xT[i][:, 0:512], in_=xbT[i * P : (i + 1) * P, 0:512]
                )
            for i in range(3):
                xq[i].dma_start(out=wv_sb[i][:], in_=wv[i * P : (i + 1) * P, :])
            for q_ in range(1, 4):
                for i in range(3):
                    xq[i].dma_start(
                        out=xT[i][:, q_ * 512 : (q_ + 1) * 512],
                        in_=xbT[i * P : (i + 1) * P, q_ * 512 : (q_ + 1) * 512],
                    )
            for p in range(2):
                nc.gpsimd.dma_start(out=wpj_sb[p][:], in_=wpj[p])

            # ---- QKV projection (chunked; most of it interleaved into the
            # attention pipeline as data arrives) ----
            QT = [cpool.tile([P, N], bf16, name=f"QT{p}", tag=f"QT{p}") for p in range(2)]
            KT = [cpool.tile([P, N], bf16, name=f"KT{p}", tag=f"KT{p}") for p in range(2)]
            # all 16 V chunks side by side: pair-production needs one copy per
            # two chunks; V chunk kc lives at cols [kc*256, (kc+1)*256)
            Vall = cpool.tile([P, NKC * HP * DHP], bf16, name="Vall", tag="Vall")
            qkv_alt = [0]  # copy-engine alternator

            def emit_qk_chunk(p, gi, j, in_attn=True):
                w_sb, dstl = ((wk_sb, KT), (wq_sb, QT))[gi]
                a = qkv_alt[0] = qkv_alt[0] + 1
                # attention-time productions use dedicated PSUM banks so they
                # never squat a sim-ring slot (that shrinks the pipeline and
                # drops the PE out of its fast p-state)
                pool, tg = (auxps, "aux") if in_attn else (simps, "sim")
                ps = pool.tile([P, 512], f32, name="qkvps", tag=tg)
                for dk in range(3):
                    nc.tensor.matmul(
                        ps[:],
                        lhsT=w_sb[dk][:, p * P : (p + 1) * P],
                        rhs=xT[dk][:, j * 512 : (j + 1) * 512],
                        start=(dk == 0),
                        stop=(dk == 2),
                    )
                if a % 2 == 0:
                    nc.vector.tensor_copy(dstl[p][:, j * 512 : (j + 1) * 512], ps[:])
                else:
                    nc.scalar.copy(dstl[p][:, j * 512 : (j + 1) * 512], ps[:])

            def emit_vpair(k2, in_attn=True):
                a = qkv_alt[0] = qkv_alt[0] + 1
                pool, tg = (auxps, "aux") if in_attn else (simps, "sim")
                ps = pool.tile([P, 2 * HP * DHP], f32, name="vps", tag=tg)
                for c in (0, 1):
                    kc = 2 * k2 + c
                    for dk in range(3):
                        nc.tensor.matmul(
                            ps[:, c * 256 : (c + 1) * 256],
                            lhsT=xT[dk][:, kc * P : (kc + 1) * P],
                            rhs=wv_sb[dk][:],
                            start=(dk == 0),
                            stop=(dk == 2),
                        )
                dst = Vall[:, k2 * 512 : (k2 + 1) * 512]
                if a % 2 == 0:
                    nc.vector.tensor_copy(dst, ps[:])
                else:
                    nc.scalar.copy(dst, ps[:])
                # ones (Z) column of each head block, at col h*64+ZOFF
                zcols = dst.rearrange("p (g c) -> p g c", c=DHP)[:, :, ZOFF : ZOFF + 1]
                nc.gpsimd.memset(zcols, 1.0)

            # preamble: productions the prologue sims need (x quarter 0 only)
            emit_qk_chunk(0, 0, 0, in_attn=False)  # KT[0] quarter 0
            emit_qk_chunk(0, 1, 0, in_attn=False)  # QT[0] query-quarter 0

            # ---- attention: 8 blocks of (pair, 512-query-range), hh-inner so
            # each x chunk feeds two consecutive half-steps ----
            OT = [cpool.tile([P, N], bf16, name=f"OT{p}", tag=f"OT{p}") for p in range(2)]
            BLOCKS8 = [(p, qq) for qq in range(4) for p in (0, 1)]
            seq = []
            for p, qq in BLOCKS8:
                for kc in range(NKC):
                    for hh in (0, 1):
                        seq.append((p, qq, hh, kc))
            nh = len(seq)  # 256
            HLOOK = 4  # half-steps the sims run ahead of the PVs

            # auto-schedule each qkv production chunk right before its first
            # consumer in the pipeline (sims run HLOOK ahead)
            side = {}

            def addside(s_, fn):
                side.setdefault(s_, []).append(fn)

            def even(i):  # iterations advance two half-steps at a time
                return max(0, (i // 2) * 2)

            first_k, first_q, first_v = {}, {}, {}
            for idx, (p, qq, hh, kc) in enumerate(seq):
                first_k.setdefault((p, kc // 4), idx)
                first_q.setdefault((p, qq), idx)
                first_v.setdefault(kc // 2, idx)
            for (p, qr), idx in first_k.items():
                if (p, qr) != (0, 0):
                    addside(even(idx - HLOOK - 1), (lambda pp, q_: lambda: emit_qk_chunk(pp, 0, q_))(p, qr))
            for (p, qj), idx in first_q.items():
                if (p, qj) != (0, 0):
                    addside(even(idx - HLOOK - 1), (lambda pp, q_: lambda: emit_qk_chunk(pp, 1, q_))(p, qj))
            for k2, idx in first_v.items():
                addside(even(idx - 2), (lambda i: lambda: emit_vpair(i))(k2))

            accs = {}
            es = {}

            def emit_sim(hi):
                p, qq, hh, kc = seq[hi]
                o = hh * DHP
                sp = simps.tile([P, 512], f32, name="sim", tag="sim")
                nc.tensor.matmul(
                    sp[:],
                    lhsT=KT[p][o : o + DHP, kc * P : (kc + 1) * P],
                    rhs=QT[p][o : o + DHP, qq * 512 : (qq + 1) * 512],
                    start=True,
                    stop=True,
                )
                e = epool.tile([P, 512], bf16, name="E", tag="E")
                # 5/8 of tiles on ACT (exact exp), 3/8 on DVE (Schraudolph):
                # leaves the DVE slack to absorb the normalize work.
                if (2 * kc + hh) % 8 in (1, 4, 6):
                    nc.vector.tensor_scalar(e[:].bitcast(i16), sp[:], SCHR_A, SCHR_B, MULT, ADD)
                else:
                    nc.scalar.activation(e[:], sp[:], EXP)
                es[hi] = e

            def emit_pv(hi):
                p, qq, hh, kc = seq[hi]
                bi = hi // 32
                o = hh * DHP
                h = p * 2 + hh
                if bi not in accs:
                    # one 1-bank accumulator per block (2-deep ring)
                    accs[bi] = accps.tile([P, 512], f32, name="acc", tag="acc")
                acc = accs[bi]
                e = es.pop(hi)
                nc.tensor.matmul(
                    acc[o : o + DHP, :],
                    lhsT=Vall[:, kc * 256 + h * DHP : kc * 256 + (h + 1) * DHP],
                    rhs=e[:],
                    start=(kc == 0),
                    stop=(kc == NKC - 1),
                )

            def norm_phase0(bi, hh, tail=False):
                # r = 1/Z for one head chunk (Z at acc row o+ZOFF), broadcast
                # to a 64-partition R tile for the multiply
                acc = accs[bi]
                o = hh * DHP
                zrow = rpool.tile([1, 512], f32, name="zrow", tag="zrow")
                if tail:  # ACT is idle at the tail; shorten the DVE chain
                    nc.scalar.copy(zrow[:], acc[o + ZOFF : o + ZOFF + 1, :])
                else:
                    nc.vector.tensor_copy(zrow[:], acc[o + ZOFF : o + ZOFF + 1, :])
                r = rpool.tile([1, 512], f32, name="r", tag="r")
                nc.vector.reciprocal_approx_fast(r[:], zrow[:])
                R = rpool.tile([DHP, 512], f32, name="R", tag="R")
                nc.gpsimd.partition_broadcast(R[:], r[:], channels=DHP)
                return R

            def norm_phase1(bi, hh, R):
                # OT[o:o+64, chunk] = acc[o:o+64, chunk] * (1/Z)
                p, qq = BLOCKS8[bi]
                o = hh * DHP
                nc.vector.tensor_mul(
                    OT[p][o : o + DHP, qq * 512 : (qq + 1) * 512],
                    accs[bi][o : o + DHP, :],
                    R[:],
                )

            def norm_chunk(bi, hh, tail=False):
                norm_phase1(bi, hh, norm_phase0(bi, hh, tail))

            # ---- output projection (partial: this core's 4 heads) ----
            def emit_proj(mc):
                yp = auxps.tile([P, D], f32, name="yp", tag="aux")
                for p in range(2):  # K=128 covers both heads of the pair
                    nc.tensor.matmul(
                        yp[:],
                        lhsT=OT[p][:, mc * P : (mc + 1) * P],
                        rhs=wpj_sb[p][:],
                        start=(p == 0),
                        stop=(p == 1),
                    )
                ys = ypool.tile([P, D], bf16, name="ys", tag="ys")
                if mc % 2 == 0:
                    nc.vector.tensor_copy(ys[:], yp[:])
                else:
                    nc.scalar.copy(ys[:], yp[:])
                if mc < 12:
                    q = (nc.sync, nc.gpsimd)[mc % 2]
                    q.dma_start(out=y[mc * P : (mc + 1) * P, :], in_=ys[:])
                else:
                    # tail: split each row-block across two of the 3 queues
                    qs = (nc.sync, nc.gpsimd, nc.scalar)
                    for half in (0, 1):
                        c0, c1 = half * 192, (half + 1) * 192
                        qs[(2 * mc + half) % 3].dma_start(
                            out=y[mc * P : (mc + 1) * P, c0:c1], in_=ys[:, c0:c1]
                        )

            proj_pending = []  # mc's ready to emit, drained 1 per 8 half-iters
            norm_pending = []  # deferred normalize chunks, same cadence

            def drain_norm():
                bi, hh, phase, R = norm_pending.pop(0)
                if phase == 0:  # recip half; queue the mul next
                    norm_pending.insert(0, (bi, hh, 1, norm_phase0(bi, hh)))
                    return
                norm_phase1(bi, hh, R)
                if not any(n[0] == bi for n in norm_pending):
                    accs.pop(bi)
                    if bi % 2 == 1 and bi < 7:
                        # both pair-blocks of query-range bi//2 are normalized
                        proj_pending.extend(range(4 * (bi // 2), 4 * (bi // 2) + 4))

            for hi in range(HLOOK):
                emit_sim(hi)
            for hp in range(0, nh, 2):
                for fn in side.pop(hp, ()):
                    fn()
                for hi in (hp + HLOOK, hp + HLOOK + 1):
                    if hi < nh:
                        emit_sim(hi)
                emit_pv(hp)
                emit_pv(hp + 1)
                if proj_pending and hp % 8 == 0:
                    emit_proj(proj_pending.pop(0))
                if norm_pending and hp % 4 == 0:
                    drain_norm()
                if hp % 32 == 30:  # block boundary
                    bi = hp // 32
                    if bi < 7:
                        # defer both chunks into the next block (the acc ring
                        # is 2 blocks deep, and bursts here knock the PE out
                        # of its fast p-state)
                        norm_pending.extend([(bi, 0, 0, None), (bi, 1, 0, None)])
                    else:
                        while norm_pending:
                            drain_norm()
                        norm_chunk(7, 0, tail=True)
                        norm_chunk(7, 1, tail=True)
                        accs.pop(7)
                        for mc in range(12, 16):
                            emit_proj(mc)

    nc.compile()
    return nc


def _prep_core_inputs(x, Wqkv, Wproj, core):
    b, hg = core // 2, core % 2
    heads = [hg * HP + i for i in range(HP)]
    xbT = np.ascontiguousarray(x[b].astype(BF16).T)
    wq = np.zeros((D, HP * DHP), np.float32)
    wk = np.zeros((D, HP * DHP), np.float32)
    wv = np.zeros((D, HP * DHP), np.float32)
    wpj = np.zeros((2, P, D), np.float32)
    for i, h in enumerate(heads):
        wq[:, i * DHP : i * DHP + DH] = Wqkv[:, h * DH : (h + 1) * DH] * SCALE
        wk[:, i * DHP : i * DHP + DH] = Wqkv[:, H * DH + h * DH : H * DH + (h + 1) * DH]
        wv_h = Wqkv[:, 2 * H * DH + h * DH : 2 * H * DH + (h + 1) * DH]
        wpj_h = Wproj[h * DH : (h + 1) * DH, :]
        # v-dims at cols [0,ZOFF) and [ZOFF+1, DH+1); ones (Z) column at ZOFF
        wv[:, i * DHP : i * DHP + ZOFF] = wv_h[:, :ZOFF]
        wv[:, i * DHP + ZOFF + 1 : i * DHP + DH + 1] = wv_h[:, ZOFF:]
        o = (i % 2) * DHP
        wpj[i // 2, o : o + ZOFF, :] = wpj_h[:ZOFF, :]
        wpj[i // 2, o + ZOFF + 1 : o + DH + 1, :] = wpj_h[ZOFF:, :]
    return {
        "xbT": xbT,
        "wq": wq.astype(BF16),
        "wk": wk.astype(BF16),
        "wv": wv.astype(BF16),
        "wpj": wpj.astype(BF16),
    }


def kernel(x, Wqkv, Wproj, bproj):
    global LAST_EXEC_NS
    if "nc" not in _CACHE:
        _CACHE["nc"] = _build_bass()
    nc = _CACHE["nc"]
    in_maps = [_prep_core_inputs(x, Wqkv, Wproj, c) for c in range(N_CORES)]
    try:
        res = run_bass_kernel_spmd(nc, in_maps, core_ids=list(range(N_CORES)))
    except Exception:
        res = run_bass_kernel_spmd(nc, in_maps, core_ids=list(range(N_CORES)))
    LAST_EXEC_NS = res.exec_time_ns
    out = np.empty((B, N, D), np.float32)
    for b in range(B):
        out[b] = res.results[2 * b]["y"].astype(np.float32) + res.results[
            2 * b + 1
        ]["y"].astype(np.float32)
    out += bproj.astype(np.float32)[None, None, :]
    return out


# revision 43
# speedup vs baseline: 1.0245x; 1.0085x over previous
"""Trainium2 Bass kernel for fused multi-head attention (B=4, N=2048, D=384, h=8, dh=48).

Sharding: 32 (batch, head) pairs across 8 cores -> core c handles batch c//2 and
heads [4*(c%2), 4*(c%2)+4). Each core computes a *partial* output projection
(its 4 heads' contribution to out @ Wproj); the host sums the two partials per
batch and adds bproj.

Per-core algorithm (everything in "transposed" layout so no PE transposes are
needed):
  xT   [384, 2048]  = x^T                          (transposed on host, bf16)
  QT   [256, 2048]  = (Wq_pad)^T @ xT  (4 heads padded dh 48->64, pair-packed)
  KT   same
  V'   [2048, 4*64] = x @ Wv_pad, with a ones-column per head at col h*64+32
                      (gpsimd memset) -> the PV matmul also accumulates the
                      softmax denominator Z for free.
  attention, software-pipelined at 512-query granularity: per (pair, q-half,
  head, key-chunk kc, j-half):
      simT[k, q512] = KT_h^T @ QT_h   (PSUM [128,512] = 1 bank, 4-deep ring;
                                       sims run 2 kc-steps ahead of PV)
      E = exp(simT)  j=0 -> ACT engine (exact exp, psum->sbuf bf16)
                     j=1 -> DVE via Schraudolph bit-trick: i16 = s*A + B
                            bitcast to bf16 ~= exp(s) (+-4% per weight, which
                            softmax-averages to ~2e-3 on the final output;
                            scores ~N(0,1) so no max subtraction needed)
      acc[o:o+64, q] += V'_kc^T @ E   (PSUM accumulate; row o+32 = Z)
  OT_h = acc[o:o+64] * (1/Z)          (DVE approx-reciprocal + gpsimd partition
                                       broadcast + DVE multiply), bf16
  y    [2048, 384]  = sum_h OT_h^T @ Wproj_h   (partial; f32 out; first half
                      interleaved into the third attention block)
"""

import os

os.environ.pop("JAX_PLATFORMS", None)  # the bass PJRT path needs the axon platform

import numpy as np
import ml_dtypes

import concourse.mybir as mybir
import concourse.tile as tile
from concourse import bacc
from concourse.bass_utils import run_bass_kernel_spmd

BF16 = ml_dtypes.bfloat16

# problem shapes (hardcoded per contract)
B, N, D = 4, 2048, 384
H, DH = 8, 48
SCALE = DH**-0.5
N_CORES = 8
HP = 4  # heads per core
DHP = 64  # padded head dim
P = 128
NKC = N // P  # 16 key-row chunks
ZOFF = 32  # partition offset of the fused softmax-denominator (Z) row within a
# head's 64-row block: engines need 32-aligned partition starts, so the ones
# column sits at col 32 of each head's V' block; v-dims occupy cols
# [0,32) and [33,49), the rest are zero. Wproj rows are laid out to match,
# with zeros at the Z/pad rows.

# Schraudolph exp for the DVE half: exp(s) ~= bitcast_bf16(int16(s*A + B)).
# A = 128/ln2; B = 127*128 + delta with delta centering the piecewise-linear
# 2^frac ~ 1+frac error (worst case ~+-4%, validated end-to-end offline).
SCHR_A = 128.0 / 0.6931471805599453
SCHR_B = 16248.87

LAST_EXEC_NS = None
_CACHE = {}


def _build_bass():
    f32 = mybir.dt.float32
    bf16 = mybir.dt.bfloat16
    i16 = mybir.dt.int16
    EXP = mybir.ActivationFunctionType.Exp
    MULT = mybir.AluOpType.mult
    ADD = mybir.AluOpType.add

    nc = bacc.Bacc("TRN2", target_bir_lowering=False, debug=False, num_devices=N_CORES)
    xbT = nc.dram_tensor("xbT", [D, N], bf16, kind="ExternalInput").ap()
    wq = nc.dram_tensor("wq", [D, HP * DHP], bf16, kind="ExternalInput").ap()
    wk = nc.dram_tensor("wk", [D, HP * DHP], bf16, kind="ExternalInput").ap()
    wv = nc.dram_tensor("wv", [D, HP * DHP], bf16, kind="ExternalInput").ap()
    wpj = nc.dram_tensor("wpj", [2, P, D], bf16, kind="ExternalInput").ap()
    # bf16 partials (summed in f32 on the host): halves the output DMA bytes
    y = nc.dram_tensor("y", [N, D], bf16, kind="ExternalOutput").ap()

    with tile.TileContext(nc) as tc:
        with (
            tc.tile_pool(name="const", bufs=1) as cpool,
            tc.tile_pool(name="epool", bufs=8) as epool,
            tc.tile_pool(name="rpool", bufs=3) as rpool,
            tc.tile_pool(name="ysb", bufs=6) as ypool,
            tc.tile_pool(name="simps", bufs=4, space="PSUM") as simps,
            tc.tile_pool(name="accps", bufs=2, space="PSUM") as accps,
            tc.tile_pool(name="auxps", bufs=2, space="PSUM") as auxps,
        ):
            # ---- load weights / x ----
            # 4-queue DMA plan (22.5 B/ns each): wk chunk + x column-halves per
            # HWDGE queue; wq (sims need it early), wv, wpj on the gpsimd
            # SWDGE queue in need order.
            wq_sb, wk_sb, wv_sb = [], [], []
            for name, srct, dst in (("wk", wk, wk_sb), ("wq", wq, wq_sb), ("wv", wv, wv_sb)):
                for i in range(3):
                    t = cpool.tile([P, HP * DHP], bf16, name=f"{name}{i}", tag=f"{name}{i}")
                    dst.append(t)
            xT = [cpool.tile([P, N], bf16, name=f"xT{i}", tag=f"xT{i}") for i in range(3)]
            wpj_sb = [
                cpool.tile([P, D], bf16, name=f"wpj{p}", tag=f"wpj{p}") for p in range(2)
            ]
            # per-dk queue: sync=dk0, scalar=dk1, gpsimd=dk2; weight chunks in
            # need order, x in quarter-column chunks so the attention pipeline
            # can start on quarter 0 while the rest streams
            xq = [nc.sync, nc.scalar, nc.gpsimd]
            for i in range(3):
                xq[i].dma_start(out=wk_sb[i][:], in_=wk[i * P : (i + 1) * P, :])
            for i in range(3):
                xq[i].dma_start(out=wq_sb[i][:], in_=wq[i * P : (i + 1) * P, :])
            for i in range(3):
                xq[i].dma_start(
                    out=xT[i][:, 0:512], in_=xbT[i * P : (i + 1) * P, 0:512]
                )
            for i in range(3):
                xq[i].dma_start(out=wv_sb[i][:], in_=wv[i * P : (i + 1) * P, :])
            for q_ in range(1, 4):
                for i in range(3):
                    xq[i].dma_start(
                        out=xT[i][:, q_ * 512 : (q_ + 1) * 512],
                        in_=xbT[i * P : (i + 1) * P, q_ * 512 : (q_ + 1) * 512],
                    )
            for p in range(2):
                nc.gpsimd.dma_start(out=wpj_sb[p][:], in_=wpj[p])

            # ---- QKV projection (chunked; most of it interleaved into the
            # attention pipeline as data arrives) ----
            QT = [cpool.tile([P, N], bf16, name=f"QT{p}", tag=f"QT{p}") for p in range(2)]
            KT = [cpool.tile([P, N], bf16, name=f"KT{p}", tag=f"KT{p}") for p in range(2)]
            # all 16 V chunks side by side: pair-production needs one copy per
            # two chunks; V chunk kc lives at cols [kc*256, (kc+1)*256)
            Vall = cpool.tile([P, NKC * HP * DHP], bf16, name="Vall", tag="Vall")
            qkv_alt = [0]  # copy-engine alternator

            def emit_qk_chunk(p, gi, j, in_attn=True):
                w_sb, dstl = ((wk_sb, KT), (wq_sb, QT))[gi]
                a = qkv_alt[0] = qkv_alt[0] + 1
                # attention-time productions use dedicated PSUM banks so they
                # never squat a sim-ring slot (that shrinks the pipeline and
                # drops the PE out of its fast p-state)
                pool, tg = (auxps, "aux") if in_attn else (simps, "sim")
                ps = pool.tile([P, 512], f32, name="qkvps", tag=tg)
                for dk in range(3):
                    nc.tensor.matmul(
                        ps[:],
                        lhsT=w_sb[dk][:, p * P : (p + 1) * P],
                        rhs=xT[dk][:, j * 512 : (j + 1) * 512],
                        start=(dk == 0),
                        stop=(dk == 2),
                    )
                if a % 2 == 0:
                    nc.vector.tensor_copy(dstl[p][:, j * 512 : (j + 1) * 512], ps[:])
                else:
                    nc.scalar.copy(dstl[p][:, j * 512 : (j + 1) * 512], ps[:])

            def emit_vpair(k2, in_attn=True):
                a = qkv_alt[0] = qkv_alt[0] + 1
                pool, tg = (auxps, "aux") if in_attn else (simps, "sim")
                ps = pool.tile([P, 2 * HP * DHP], f32, name="vps", tag=tg)
                for c in (0, 1):
                    kc = 2 * k2 + c
                    for dk in range(3):
                        nc.tensor.matmul(
                            ps[:, c * 256 : (c + 1) * 256],
                            lhsT=xT[dk][:, kc * P : (kc + 1) * P],
                            rhs=wv_sb[dk][:],
                            start=(dk == 0),
                            stop=(dk == 2),
                        )
                dst = Vall[:, k2 * 512 : (k2 + 1) * 512]
                if a % 2 == 0:
                    nc.vector.tensor_copy(dst, ps[:])
                else:
                    nc.scalar.copy(dst, ps[:])
                # ones (Z) column of each head block, at col h*64+ZOFF
                zcols = dst.rearrange("p (g c) -> p g c", c=DHP)[:, :, ZOFF : ZOFF + 1]
                nc.gpsimd.memset(zcols, 1.0)

            # preamble: productions the prologue sims need (x quarter 0 only)
            emit_qk_chunk(0, 0, 0, in_attn=False)  # KT[0] quarter 0
            emit_qk_chunk(0, 1, 0, in_attn=False)  # QT[0] query-quarter 0

            # ---- attention: 8 blocks of (pair, 512-query-range), hh-inner so
            # each x chunk feeds two consecutive half-steps ----
            OT = [cpool.tile([P, N], bf16, name=f"OT{p}", tag=f"OT{p}") for p in range(2)]
            BLOCKS8 = [(p, qq) for qq in range(4) for p in (0, 1)]
            seq = []
            for p, qq in BLOCKS8:
                for kc in range(NKC):
                    for hh in (0, 1):
                        seq.append((p, qq, hh, kc))
            nh = len(seq)  # 256
            HLOOK = 4  # half-steps the sims run ahead of the PVs

            # auto-schedule each qkv production chunk right before its first
            # consumer in the pipeline (sims run HLOOK ahead)
            side = {}

            def addside(s_, fn):
                side.setdefault(s_, []).append(fn)

            def even(i):  # iterations advance two half-steps at a time
                return max(0, (i // 2) * 2)

            first_k, first_q, first_v = {}, {}, {}
            for idx, (p, qq, hh, kc) in enumerate(seq):
                first_k.setdefault((p, kc // 4), idx)
                first_q.setdefault((p, qq), idx)
                first_v.setdefault(kc // 2, idx)
            for (p, qr), idx in first_k.items():
                if (p, qr) != (0, 0):
                    addside(even(idx - HLOOK - 1), (lambda pp, q_: lambda: emit_qk_chunk(pp, 0, q_))(p, qr))
            for (p, qj), idx in first_q.items():
                if (p, qj) != (0, 0):
                    addside(even(idx - HLOOK - 1), (lambda pp, q_: lambda: emit_qk_chunk(pp, 1, q_))(p, qj))
            for k2, idx in first_v.items():
                addside(even(idx - 2), (lambda i: lambda: emit_vpair(i))(k2))

            accs = {}
            es = {}

            def emit_sim(hi):
                p, qq, hh, kc = seq[hi]
                o = hh * DHP
                sp = simps.tile([P, 512], f32, name="sim", tag="sim")
                nc.tensor.matmul(
                    sp[:],
                    lhsT=KT[p][o : o + DHP, kc * P : (kc + 1) * P],
                    rhs=QT[p][o : o + DHP, qq * 512 : (qq + 1) * 512],
                    start=True,
                    stop=True,
                )
                e = epool.tile([P, 512], bf16, name="E", tag="E")
                # 5/8 of tiles on ACT (exact exp), 3/8 on DVE (Schraudolph):
                # leaves the DVE slack to absorb the normalize work.
                if (2 * kc + hh) % 8 in (1, 4, 6):
                    nc.vector.tensor_scalar(e[:].bitcast(i16), sp[:], SCHR_A, SCHR_B, MULT, ADD)
                else:
                    nc.scalar.activation(e[:], sp[:], EXP)
                es[hi] = e

            def emit_pv(hi):
                p, qq, hh, kc = seq[hi]
                bi = hi // 32
                o = hh * DHP
                h = p * 2 + hh
                if bi not in accs:
                    # one 1-bank accumulator per block (2-deep ring)
                    accs[bi] = accps.tile([P, 512], f32, name="acc", tag="acc")
                acc = accs[bi]
                e = es.pop(hi)
                nc.tensor.matmul(
                    acc[o : o + DHP, :],
                    lhsT=Vall[:, kc * 256 + h * DHP : kc * 256 + (h + 1) * DHP],
                    rhs=e[:],
                    start=(kc == 0),
                    stop=(kc == NKC - 1),
                )

            def norm_phase0(bi, hh, tail=False):
                # r = 1/Z for one head chunk (Z at acc row o+ZOFF), broadcast
                # to a 64-partition R tile for the multiply
                acc = accs[bi]
                o = hh * DHP
                zrow = rpool.tile([1, 512], f32, name="zrow", tag="zrow")
                if tail:  # ACT is idle at the tail; shorten the DVE chain
                    nc.scalar.copy(zrow[:], acc[o + ZOFF : o + ZOFF + 1, :])
                else:
                    nc.vector.tensor_copy(zrow[:], acc[o + ZOFF : o + ZOFF + 1, :])
                r = rpool.tile([1, 512], f32, name="r", tag="r")
                nc.vector.reciprocal_approx_fast(r[:], zrow[:])
                R = rpool.tile([DHP, 512], f32, name="R", tag="R")
                nc.gpsimd.partition_broadcast(R[:], r[:], channels=DHP)
                return R

            def norm_phase1(bi, hh, R):
                # OT[o:o+64, chunk] = acc[o:o+64, chunk] * (1/Z)
                p, qq = BLOCKS8[bi]
                o = hh * DHP
                nc.vector.tensor_mul(
                    OT[p][o : o + DHP, qq * 512 : (qq + 1) * 512],
                    accs[bi][o : o + DHP, :],
                    R[:],
                )

            def norm_chunk(bi, hh, tail=False):
                norm_phase1(bi, hh, norm_phase0(bi, hh, tail))

            # ---- output projection (partial: this core's 4 heads) ----
            def emit_proj(mc):
                yp = auxps.tile([P, D], f32, name="yp", tag="aux")
                for p in range(2):  # K=128 covers both heads of the pair
                    nc.tensor.matmul(
                        yp[:],
                        lhsT=OT[p][:, mc * P : (mc + 1) * P],
                        rhs=wpj_sb[p][:],
                        start=(p == 0),
                        stop=(p == 1),
                    )
                ys = ypool.tile([P, D], bf16, name="ys", tag="ys")
                if mc % 2 == 0:
                    nc.vector.tensor_copy(ys[:], yp[:])
                else:
                    nc.scalar.copy(ys[:], yp[:])
                if mc < 12:
                    q = (nc.sync, nc.gpsimd)[mc % 2]
                    q.dma_start(out=y[mc * P : (mc + 1) * P, :], in_=ys[:])
                else:
                    # tail: split each row-block across two of the 3 queues
                    qs = (nc.sync, nc.gpsimd, nc.scalar)
                    for half in (0, 1):
                        c0, c1 = half * 192, (half + 1) * 192
                        qs[(2 * mc + half) % 3].dma_start(
                            out=y[mc * P : (mc + 1) * P, c0:c1], in_=ys[:, c0:c1]
                        )

            proj_pending = []  # mc's ready to emit, drained 1 per 8 half-iters
            norm_pending = []  # deferred normalize chunks, same cadence

            def drain_norm():
                bi, hh, phase, R = norm_pending.pop(0)
                if phase == 0:  # recip half; queue the mul next
                    norm_pending.insert(0, (bi, hh, 1, norm_phase0(bi, hh)))
                    return
                norm_phase1(bi, hh, R)
                if not any(n[0] == bi for n in norm_pending):
                    accs.pop(bi)
                    if bi % 2 == 1 and bi < 7:
                        # both pair-blocks of query-range bi//2 are normalized
                        proj_pending.extend(range(4 * (bi // 2), 4 * (bi // 2) + 4))

            for hi in range(HLOOK):
                emit_sim(hi)
            for hp in range(0, nh, 2):
                for fn in side.pop(hp, ()):
                    fn()
                for hi in (hp + HLOOK, hp + HLOOK + 1):
                    if hi < nh:
                        emit_sim(hi)
                emit_pv(hp)
                emit_pv(hp + 1)
                if proj_pending and hp % 8 == 2:
                    emit_proj(proj_pending.pop(0))
                if norm_pending and hp % 4 == 0:
                    drain_norm()
                if hp % 32 == 30:  # block boundary
                    bi = hp // 32
                    if bi < 7:
                        # defer both chunks into the next block (the acc ring
                        # is 2 blocks deep, and bursts here knock the PE out
                        # of its fast p-state)
                        norm_pending.extend([(bi, 0, 0, None), (bi, 1, 0, None)])
                    else:
                        while norm_pending:
                            drain_norm()
                        norm_chunk(7, 0, tail=True)
                        norm_chunk(7, 1, tail=True)
                        accs.pop(7)
                        for mc in range(12, 16):
                            emit_proj(mc)

    nc.compile()
    return nc


def _prep_core_inputs(x, Wqkv, Wproj, core):
    b, hg = core // 2, core % 2
    heads = [hg * HP + i for i in range(HP)]
    xbT = np.ascontiguousarray(x[b].astype(BF16).T)
    wq = np.zeros((D, HP * DHP), np.float32)
    wk = np.zeros((D, HP * DHP), np.float32)
    wv = np.zeros((D, HP * DHP), np.float32)
    wpj = np.zeros((2, P, D), np.float32)
    for i, h in enumerate(heads):
        wq[:, i * DHP : i * DHP + DH] = Wqkv[:, h * DH : (h + 1) * DH] * SCALE
        wk[:, i * DHP : i * DHP + DH] = Wqkv[:, H * DH + h * DH : H * DH + (h + 1) * DH]
        wv_h = Wqkv[:, 2 * H * DH + h * DH : 2 * H * DH + (h + 1) * DH]
        wpj_h = Wproj[h * DH : (h + 1) * DH, :]
        # v-dims at cols [0,ZOFF) and [ZOFF+1, DH+1); ones (Z) column at ZOFF
        wv[:, i * DHP : i * DHP + ZOFF] = wv_h[:, :ZOFF]
        wv[:, i * DHP + ZOFF + 1 : i * DHP + DH + 1] = wv_h[:, ZOFF:]
        o = (i % 2) * DHP
        wpj[i // 2, o : o + ZOFF, :] = wpj_h[:ZOFF, :]
        wpj[i // 2, o + ZOFF + 1 : o + DH + 1, :] = wpj_h[ZOFF:, :]
    return {
        "xbT": xbT,
        "wq": wq.astype(BF16),
        "wk": wk.astype(BF16),
        "wv": wv.astype(BF16),
        "wpj": wpj.astype(BF16),
    }


def kernel(x, Wqkv, Wproj, bproj):
    global LAST_EXEC_NS
    if "nc" not in _CACHE:
        _CACHE["nc"] = _build_bass()
    nc = _CACHE["nc"]
    in_maps = [_prep_core_inputs(x, Wqkv, Wproj, c) for c in range(N_CORES)]
    try:
        res = run_bass_kernel_spmd(nc, in_maps, core_ids=list(range(N_CORES)))
    except Exception:
        res = run_bass_kernel_spmd(nc, in_maps, core_ids=list(range(N_CORES)))
    LAST_EXEC_NS = res.exec_time_ns
    out = np.empty((B, N, D), np.float32)
    for b in range(B):
        out[b] = res.results[2 * b]["y"].astype(np.float32) + res.results[
            2 * b + 1
        ]["y"].astype(np.float32)
    out += bproj.astype(np.float32)[None, None, :]
    return out


# revision 44
# speedup vs baseline: 1.0269x; 1.0024x over previous
"""Trainium2 Bass kernel for fused multi-head attention (B=4, N=2048, D=384, h=8, dh=48).

Sharding: 32 (batch, head) pairs across 8 cores -> core c handles batch c//2 and
heads [4*(c%2), 4*(c%2)+4). Each core computes a *partial* output projection
(its 4 heads' contribution to out @ Wproj) in bf16; the host sums the two
partials per batch in f32 and adds bproj.

Per-core algorithm (everything in "transposed" layout so no PE transposes are
needed):
  xT   [384, 2048]  = x^T     (bf16, streamed in column-quarters over the 3 DMA
                               queues so compute starts at ~12us, not ~33us)
  QT   [256, 2048]  = (Wq_pad)^T @ xT  (4 heads padded dh 48->64, pair-packed;
                               produced in 512-column chunks auto-scheduled
                               just before their first consumer)
  KT   same; V' chunks of x @ Wv_pad with a ones-column per head at col
       kc*256 + h*64+32 -> the PV matmul accumulates the softmax denominator
       Z for free.

  attention: 256 half-steps over 8 blocks of (pair, 512-query-range), hh inner
  so each arriving x chunk feeds two consecutive half-steps. Per half-step:
      simT[k, q512] = KT_h^T @ QT_h   (PSUM [128,512] = 1 bank, 4-slot ring;
                                       sims run 4 half-steps ahead of the PVs)
      E = exp(simT)   5/8 on ACT (exact exp); 3/8 on DVE via the Schraudolph
                      bit-trick i16 = s*A + B bitcast to bf16 (+-4% per weight,
                      which softmax-averages to ~1e-3 on the final output;
                      scores ~N(0,1) so no max subtraction is needed)
      acc[o:o+64, :] += V'_kc^T @ E   (one 1-bank accumulator per block,
                                       2-slot ring; row o+32 accumulates Z)
  normalize: OT = acc * (1/Z) (DVE approx-reciprocal + gpsimd partition
  broadcast + DVE multiply), deferred into the next block and drained one
  sub-op per 4 half-steps - bursts here knock the PE out of its fast p-state,
  which costs 2x on every matmul.
  proj: y[mc*128 : ...] = sum_p OT_p^T @ Wproj_p per 128-row chunk, emitted as
  soon as its query-range is normalized (only the last 4 chunks trail the
  attention); PSUM for proj/productions comes from a dedicated 2-slot aux pool
  so they never squat a sim-ring slot.
"""

import os

os.environ.pop("JAX_PLATFORMS", None)  # the bass PJRT path needs the axon platform

import numpy as np
import ml_dtypes

import concourse.mybir as mybir
import concourse.tile as tile
from concourse import bacc
from concourse.bass_utils import run_bass_kernel_spmd

BF16 = ml_dtypes.bfloat16

# problem shapes (hardcoded per contract)
B, N, D = 4, 2048, 384
H, DH = 8, 48
SCALE = DH**-0.5
N_CORES = 8
HP = 4  # heads per core
DHP = 64  # padded head dim
P = 128
NKC = N // P  # 16 key-row chunks
ZOFF = 32  # partition offset of the fused softmax-denominator (Z) row within a
# head's 64-row block: engines need 32-aligned partition starts, so the ones
# column sits at col 32 of each head's V' block; v-dims occupy cols
# [0,32) and [33,49), the rest are zero. Wproj rows are laid out to match,
# with zeros at the Z/pad rows.

# Schraudolph exp for the DVE half: exp(s) ~= bitcast_bf16(int16(s*A + B)).
# A = 128/ln2; B = 127*128 + delta with delta centering the piecewise-linear
# 2^frac ~ 1+frac error (worst case ~+-4%, validated end-to-end offline).
SCHR_A = 128.0 / 0.6931471805599453
SCHR_B = 16248.87

LAST_EXEC_NS = None
_CACHE = {}


def _build_bass():
    f32 = mybir.dt.float32
    bf16 = mybir.dt.bfloat16
    i16 = mybir.dt.int16
    EXP = mybir.ActivationFunctionType.Exp
    MULT = mybir.AluOpType.mult
    ADD = mybir.AluOpType.add

    nc = bacc.Bacc("TRN2", target_bir_lowering=False, debug=False, num_devices=N_CORES)
    xbT = nc.dram_tensor("xbT", [D, N], bf16, kind="ExternalInput").ap()
    wq = nc.dram_tensor("wq", [D, HP * DHP], bf16, kind="ExternalInput").ap()
    wk = nc.dram_tensor("wk", [D, HP * DHP], bf16, kind="ExternalInput").ap()
    wv = nc.dram_tensor("wv", [D, HP * DHP], bf16, kind="ExternalInput").ap()
    wpj = nc.dram_tensor("wpj", [2, P, D], bf16, kind="ExternalInput").ap()
    # bf16 partials (summed in f32 on the host): halves the output DMA bytes
    y = nc.dram_tensor("y", [N, D], bf16, kind="ExternalOutput").ap()

    with tile.TileContext(nc) as tc:
        with (
            tc.tile_pool(name="const", bufs=1) as cpool,
            tc.tile_pool(name="epool", bufs=8) as epool,
            tc.tile_pool(name="rpool", bufs=3) as rpool,
            tc.tile_pool(name="ysb", bufs=6) as ypool,
            tc.tile_pool(name="simps", bufs=4, space="PSUM") as simps,
            tc.tile_pool(name="accps", bufs=2, space="PSUM") as accps,
            tc.tile_pool(name="auxps", bufs=2, space="PSUM") as auxps,
        ):
            # ---- load weights / x ----
            # 4-queue DMA plan (22.5 B/ns each): wk chunk + x column-halves per
            # HWDGE queue; wq (sims need it early), wv, wpj on the gpsimd
            # SWDGE queue in need order.
            wq_sb, wk_sb, wv_sb = [], [], []
            for name, srct, dst in (("wk", wk, wk_sb), ("wq", wq, wq_sb), ("wv", wv, wv_sb)):
                for i in range(3):
                    t = cpool.tile([P, HP * DHP], bf16, name=f"{name}{i}", tag=f"{name}{i}")
                    dst.append(t)
            xT = [cpool.tile([P, N], bf16, name=f"xT{i}", tag=f"xT{i}") for i in range(3)]
            wpj_sb = [
                cpool.tile([P, D], bf16, name=f"wpj{p}", tag=f"wpj{p}") for p in range(2)
            ]
            # per-dk queue: sync=dk0, scalar=dk1, gpsimd=dk2; weight chunks in
            # need order, x in quarter-column chunks so the attention pipeline
            # can start on quarter 0 while the rest streams
            xq = [nc.sync, nc.scalar, nc.gpsimd]
            for i in range(3):
                xq[i].dma_start(out=wk_sb[i][:], in_=wk[i * P : (i + 1) * P, :])
            for i in range(3):
                xq[i].dma_start(out=wq_sb[i][:], in_=wq[i * P : (i + 1) * P, :])
            for i in range(3):
                xq[i].dma_start(
                    out=xT[i][:, 0:512], in_=xbT[i * P : (i + 1) * P, 0:512]
                )
            for i in range(3):
                xq[i].dma_start(out=wv_sb[i][:], in_=wv[i * P : (i + 1) * P, :])
            for q_ in range(1, 4):
                for i in range(3):
                    xq[i].dma_start(
                        out=xT[i][:, q_ * 512 : (q_ + 1) * 512],
                        in_=xbT[i * P : (i + 1) * P, q_ * 512 : (q_ + 1) * 512],
                    )
            for p in range(2):
                nc.gpsimd.dma_start(out=wpj_sb[p][:], in_=wpj[p])

            # ---- QKV projection (chunked; most of it interleaved into the
            # attention pipeline as data arrives) ----
            QT = [cpool.tile([P, N], bf16, name=f"QT{p}", tag=f"QT{p}") for p in range(2)]
            KT = [cpool.tile([P, N], bf16, name=f"KT{p}", tag=f"KT{p}") for p in range(2)]
            # all 16 V chunks side by side: pair-production needs one copy per
            # two chunks; V chunk kc lives at cols [kc*256, (kc+1)*256)
            Vall = cpool.tile([P, NKC * HP * DHP], bf16, name="Vall", tag="Vall")
            qkv_alt = [0]  # copy-engine alternator

            def emit_qk_chunk(p, gi, j, in_attn=True):
                w_sb, dstl = ((wk_sb, KT), (wq_sb, QT))[gi]
                a = qkv_alt[0] = qkv_alt[0] + 1
                # attention-time productions use dedicated PSUM banks so they
                # never squat a sim-ring slot (that shrinks the pipeline and
                # drops the PE out of its fast p-state)
                pool, tg = (auxps, "aux") if in_attn else (simps, "sim")
                ps = pool.tile([P, 512], f32, name="qkvps", tag=tg)
                for dk in range(3):
                    nc.tensor.matmul(
                        ps[:],
                        lhsT=w_sb[dk][:, p * P : (p + 1) * P],
                        rhs=xT[dk][:, j * 512 : (j + 1) * 512],
                        start=(dk == 0),
                        stop=(dk == 2),
                    )
                if a % 2 == 0:
                    nc.vector.tensor_copy(dstl[p][:, j * 512 : (j + 1) * 512], ps[:])
                else:
                    nc.scalar.copy(dstl[p][:, j * 512 : (j + 1) * 512], ps[:])

            def emit_vpair(k2, in_attn=True):
                a = qkv_alt[0] = qkv_alt[0] + 1
                pool, tg = (auxps, "aux") if in_attn else (simps, "sim")
                ps = pool.tile([P, 2 * HP * DHP], f32, name="vps", tag=tg)
                for c in (0, 1):
                    kc = 2 * k2 + c
                    for dk in range(3):
                        nc.tensor.matmul(
                            ps[:, c * 256 : (c + 1) * 256],
                            lhsT=xT[dk][:, kc * P : (kc + 1) * P],
                            rhs=wv_sb[dk][:],
                            start=(dk == 0),
                            stop=(dk == 2),
                        )
                dst = Vall[:, k2 * 512 : (k2 + 1) * 512]
                if a % 2 == 0:
                    nc.vector.tensor_copy(dst, ps[:])
                else:
                    nc.scalar.copy(dst, ps[:])
                # ones (Z) column of each head block, at col h*64+ZOFF
                zcols = dst.rearrange("p (g c) -> p g c", c=DHP)[:, :, ZOFF : ZOFF + 1]
                nc.gpsimd.memset(zcols, 1.0)

            # preamble: productions the prologue sims need (x quarter 0 only)
            emit_qk_chunk(0, 0, 0, in_attn=False)  # KT[0] quarter 0
            emit_qk_chunk(0, 1, 0, in_attn=False)  # QT[0] query-quarter 0

            # ---- attention: 8 blocks of (pair, 512-query-range), hh-inner so
            # each x chunk feeds two consecutive half-steps ----
            OT = [cpool.tile([P, N], bf16, name=f"OT{p}", tag=f"OT{p}") for p in range(2)]
            BLOCKS8 = [(p, qq) for qq in range(4) for p in (0, 1)]
            seq = []
            for p, qq in BLOCKS8:
                for kc in range(NKC):
                    for hh in (0, 1):
                        seq.append((p, qq, hh, kc))
            nh = len(seq)  # 256
            HLOOK = 4  # half-steps the sims run ahead of the PVs

            # auto-schedule each qkv production chunk right before its first
            # consumer in the pipeline (sims run HLOOK ahead)
            side = {}

            def addside(s_, fn):
                side.setdefault(s_, []).append(fn)

            def even(i):  # iterations advance two half-steps at a time
                return max(0, (i // 2) * 2)

            first_k, first_q, first_v = {}, {}, {}
            for idx, (p, qq, hh, kc) in enumerate(seq):
                first_k.setdefault((p, kc // 4), idx)
                first_q.setdefault((p, qq), idx)
                first_v.setdefault(kc // 2, idx)
            for (p, qr), idx in first_k.items():
                if (p, qr) != (0, 0):
                    addside(even(idx - HLOOK - 1), (lambda pp, q_: lambda: emit_qk_chunk(pp, 0, q_))(p, qr))
            for (p, qj), idx in first_q.items():
                if (p, qj) != (0, 0):
                    addside(even(idx - HLOOK - 1), (lambda pp, q_: lambda: emit_qk_chunk(pp, 1, q_))(p, qj))
            for k2, idx in first_v.items():
                addside(even(idx - 2), (lambda i: lambda: emit_vpair(i))(k2))

            accs = {}
            es = {}

            def emit_sim(hi):
                p, qq, hh, kc = seq[hi]
                o = hh * DHP
                sp = simps.tile([P, 512], f32, name="sim", tag="sim")
                nc.tensor.matmul(
                    sp[:],
                    lhsT=KT[p][o : o + DHP, kc * P : (kc + 1) * P],
                    rhs=QT[p][o : o + DHP, qq * 512 : (qq + 1) * 512],
                    start=True,
                    stop=True,
                )
                e = epool.tile([P, 512], bf16, name="E", tag="E")
                # 5/8 of tiles on ACT (exact exp), 3/8 on DVE (Schraudolph):
                # leaves the DVE slack to absorb the normalize work.
                if (2 * kc + hh) % 8 in (1, 4, 6):
                    nc.vector.tensor_scalar(e[:].bitcast(i16), sp[:], SCHR_A, SCHR_B, MULT, ADD)
                else:
                    nc.scalar.activation(e[:], sp[:], EXP)
                es[hi] = e

            def emit_pv(hi):
                p, qq, hh, kc = seq[hi]
                bi = hi // 32
                o = hh * DHP
                h = p * 2 + hh
                if bi not in accs:
                    # one 1-bank accumulator per block (2-deep ring)
                    accs[bi] = accps.tile([P, 512], f32, name="acc", tag="acc")
                acc = accs[bi]
                e = es.pop(hi)
                nc.tensor.matmul(
                    acc[o : o + DHP, :],
                    lhsT=Vall[:, kc * 256 + h * DHP : kc * 256 + (h + 1) * DHP],
                    rhs=e[:],
                    start=(kc == 0),
                    stop=(kc == NKC - 1),
                )

            def norm_phase0(bi, hh, tail=False):
                # r = 1/Z for one head chunk (Z at acc row o+ZOFF), broadcast
                # to a 64-partition R tile for the multiply
                acc = accs[bi]
                o = hh * DHP
                zrow = rpool.tile([1, 512], f32, name="zrow", tag="zrow")
                if tail:  # ACT is idle at the tail; shorten the DVE chain
                    nc.scalar.copy(zrow[:], acc[o + ZOFF : o + ZOFF + 1, :])
                else:
                    nc.vector.tensor_copy(zrow[:], acc[o + ZOFF : o + ZOFF + 1, :])
                r = rpool.tile([1, 512], f32, name="r", tag="r")
                nc.vector.reciprocal_approx_fast(r[:], zrow[:])
                R = rpool.tile([DHP, 512], f32, name="R", tag="R")
                nc.gpsimd.partition_broadcast(R[:], r[:], channels=DHP)
                return R

            def norm_phase1(bi, hh, R):
                # OT[o:o+64, chunk] = acc[o:o+64, chunk] * (1/Z)
                p, qq = BLOCKS8[bi]
                o = hh * DHP
                nc.vector.tensor_mul(
                    OT[p][o : o + DHP, qq * 512 : (qq + 1) * 512],
                    accs[bi][o : o + DHP, :],
                    R[:],
                )

            def norm_chunk(bi, hh, tail=False):
                norm_phase1(bi, hh, norm_phase0(bi, hh, tail))

            # ---- output projection (partial: this core's 4 heads) ----
            def emit_proj(mc):
                yp = auxps.tile([P, D], f32, name="yp", tag="aux")
                for p in range(2):  # K=128 covers both heads of the pair
                    nc.tensor.matmul(
                        yp[:],
                        lhsT=OT[p][:, mc * P : (mc + 1) * P],
                        rhs=wpj_sb[p][:],
                        start=(p == 0),
                        stop=(p == 1),
                    )
                ys = ypool.tile([P, D], bf16, name="ys", tag="ys")
                if mc % 2 == 0:
                    nc.vector.tensor_copy(ys[:], yp[:])
                else:
                    nc.scalar.copy(ys[:], yp[:])
                if mc < 12:
                    q = (nc.sync, nc.gpsimd)[mc % 2]
                    q.dma_start(out=y[mc * P : (mc + 1) * P, :], in_=ys[:])
                else:
                    # tail: split each row-block across two of the 3 queues
                    qs = (nc.sync, nc.gpsimd, nc.scalar)
                    for half in (0, 1):
                        c0, c1 = half * 192, (half + 1) * 192
                        qs[(2 * mc + half) % 3].dma_start(
                            out=y[mc * P : (mc + 1) * P, c0:c1], in_=ys[:, c0:c1]
                        )

            proj_pending = []  # mc's ready to emit, drained 1 per 8 half-iters
            norm_pending = []  # deferred normalize chunks, same cadence

            def drain_norm():
                bi, hh, phase, R = norm_pending.pop(0)
                if phase == 0:  # recip half; queue the mul next
                    norm_pending.insert(0, (bi, hh, 1, norm_phase0(bi, hh)))
                    return
                norm_phase1(bi, hh, R)
                if not any(n[0] == bi for n in norm_pending):
                    accs.pop(bi)
                    if bi % 2 == 1 and bi < 7:
                        # both pair-blocks of query-range bi//2 are normalized
                        proj_pending.extend(range(4 * (bi // 2), 4 * (bi // 2) + 4))

            for hi in range(HLOOK):
                emit_sim(hi)
            for hp in range(0, nh, 2):
                for fn in side.pop(hp, ()):
                    fn()
                for hi in (hp + HLOOK, hp + HLOOK + 1):
                    if hi < nh:
                        emit_sim(hi)
                emit_pv(hp)
                emit_pv(hp + 1)
                if proj_pending and hp % 8 == 2:
                    emit_proj(proj_pending.pop(0))
                if norm_pending and hp % 4 == 0:
                    drain_norm()
                if hp % 32 == 30:  # block boundary
                    bi = hp // 32
                    if bi < 7:
                        # defer both chunks into the next block (the acc ring
                        # is 2 blocks deep, and bursts here knock the PE out
                        # of its fast p-state)
                        norm_pending.extend([(bi, 0, 0, None), (bi, 1, 0, None)])
                    else:
                        while norm_pending:
                            drain_norm()
                        norm_chunk(7, 0, tail=True)
                        norm_chunk(7, 1, tail=True)
                        accs.pop(7)
                        for mc in range(12, 16):
                            emit_proj(mc)

    nc.compile()
    return nc


def _prep_core_inputs(x, Wqkv, Wproj, core):
    b, hg = core // 2, core % 2
    heads = [hg * HP + i for i in range(HP)]
    xbT = np.ascontiguousarray(x[b].astype(BF16).T)
    wq = np.zeros((D, HP * DHP), np.float32)
    wk = np.zeros((D, HP * DHP), np.float32)
    wv = np.zeros((D, HP * DHP), np.float32)
    wpj = np.zeros((2, P, D), np.float32)
    for i, h in enumerate(heads):
        wq[:, i * DHP : i * DHP + DH] = Wqkv[:, h * DH : (h + 1) * DH] * SCALE
        wk[:, i * DHP : i * DHP + DH] = Wqkv[:, H * DH + h * DH : H * DH + (h + 1) * DH]
        wv_h = Wqkv[:, 2 * H * DH + h * DH : 2 * H * DH + (h + 1) * DH]
        wpj_h = Wproj[h * DH : (h + 1) * DH, :]
        # v-dims at cols [0,ZOFF) and [ZOFF+1, DH+1); ones (Z) column at ZOFF
        wv[:, i * DHP : i * DHP + ZOFF] = wv_h[:, :ZOFF]
        wv[:, i * DHP + ZOFF + 1 : i * DHP + DH + 1] = wv_h[:, ZOFF:]
        o = (i % 2) * DHP
        wpj[i // 2, o : o + ZOFF, :] = wpj_h[:ZOFF, :]
        wpj[i // 2, o + ZOFF + 1 : o + DH + 1, :] = wpj_h[ZOFF:, :]
    return {
        "xbT": xbT,
        "wq": wq.astype(BF16),
        "wk": wk.astype(BF16),
        "wv": wv.astype(BF16),
        "wpj": wpj.astype(BF16),
    }


def kernel(x, Wqkv, Wproj, bproj):
    global LAST_EXEC_NS
    if "nc" not in _CACHE:
        _CACHE["nc"] = _build_bass()
    nc = _CACHE["nc"]
    in_maps = [_prep_core_inputs(x, Wqkv, Wproj, c) for c in range(N_CORES)]
    try:
        res = run_bass_kernel_spmd(nc, in_maps, core_ids=list(range(N_CORES)))
    except Exception:
        res = run_bass_kernel_spmd(nc, in_maps, core_ids=list(range(N_CORES)))
    LAST_EXEC_NS = res.exec_time_ns
    out = np.empty((B, N, D), np.float32)
    for b in range(B):
        out[b] = res.results[2 * b]["y"].astype(np.float32) + res.results[
            2 * b + 1
        ]["y"].astype(np.float32)
    out += bproj.astype(np.float32)[None, None, :]
    return out
